# revision 19
# baseline (speedup 1.0000x reference)
"""ConvLSTM (pixel-wise, 1x1 convs) Trainium2 Bass kernel.

Math (after exact algebraic folding):
  per pixel, per t:  g1 = W1x @ x_t + W1h @ h1 + b1   (W1x = Wih1 @ (W_red * denorm_scale))
                     i,f,g,o = split(g1); c1 = sig(f)*c1 + sig(i)*tanh(g); h1 = sig(o)*tanh(c1)
                     g2 = W21 @ h1 + W22 @ h2 + b2    (W21 = Wih2 @ Wc1)
                     c2,h2 analogous
  out = (W_head @ Wc2) @ h2_final + const

Sharding: batch b -> core b (8 cores, no collectives).

Per-core layout (one chunk = all 16384 pixels):
  S1[0..2] [128, HW] bf16 (rotating mod 3 by t): rows 0:64 h1(t-1),
      rows 64:92 x(t), row 92 ones (shipped as a 29th x channel).
      One K=93 matmul per gate computes W1h@h1 + W1x@x + b1.
      3-deep rotation lets the x(t) DMA start a full timestep early
      (it only WARs against cell1 reads of t-3), hiding the ~36us
      29-partition-bound transfer that otherwise stalls ~19.5us/step.
  V [128, HW] bf16: rows 0:64 h1(t) (same data as S1next rows 0:64,
      placed twice), rows 64:128 h2. Cell2 gates are ONE K=128 matmul
      per gate-half -- W21@h1 + W22@h2 fused via stacked weights
      (w2 = [W21.T; W22.T]), halving cell2 PE time vs the two-mm
      accumulation scheme (PE streams N cols per mm regardless of K).
  c1/c2 [128, HALF]: A-half pixels on partitions 0:64, B-half on 64:128.
      bf16 by default: keeps every pointwise STT in the DVE 2x_1P perf
      mode (~424ns vs ~690ns for an f32 operand at 1x).
  Gates per cell land in ONE [128, 4F] PSUM tile in order (i, f, o, g).
  gmode=vtanh: sigma(x) = (tanh(x/2)+1)/2 with the 1/2 folded into the
      gate weights and doubled c/h state folded into consumer weights, so
      ONE Tanh instruction covers all four gates (ACT inst fixed cost is
      ~352 cyc; also bf16-safe -- only near-zero tanh values are stored).
  Pointwise runs on DVE via fused scalar_tensor_tensor ((a op s) op b);
  h is built once in a [128, F] plane then placed with 4x-mode bf16
  copies (h1 goes to BOTH S1next and V rows 0:64; h2 to V rows 64:128).
  ACT (tanh at 1 elem/cyc/lane, 1.2 GHz) is the pacing engine at
  ~5.4us/slot; PE ~3.5us (cold 1.2 GHz -- HAM never unthrottles in this
  environment), DVE ~4.6us.
  NOTE (hardware, verified by minimal repro in a prior session): an
  accumulating matmul pair with K<=64 at row bases 0 and 64 into one
  PSUM region makes walrus pick row-tiling and faults the device. The
  single K=128 cell2 mm avoids accumulation pairs entirely.
"""

import numpy as np
import ml_dtypes

import concourse.bass as bass
import concourse.tile as tile
from concourse import bacc, mybir
from concourse.bass_utils import run_bass_kernel_spmd

F32 = mybir.dt.float32
BF16 = mybir.dt.bfloat16
AF = mybir.ActivationFunctionType

T, CIN, HID = 8, 28, 64
H = W = 128
HW = H * W            # pixels per core (one batch element)
HALF = HW // 2
NCORES = 8
K1 = HID + CIN + 1    # 93: h1 rows, x rows, ones row

import os
CFG = dict(
    fd=512,            # pixels per half per group (psum: 2 cells x [128, 4*fd] fp32)
    c_dtype="bf16",    # c-state dtype: "bf16" (DVE 2x mode) | "f32" (exact-ish)
    s1_depth=3,        # S1 tile rotation depth (3 hides the x DMA fully;
                       # forced to 2 when c_dtype=f32 to fit SBUF)
    plane_bufs=2,
    gmode="vtanh",     # see module docstring; "tanh" = exact sigmoid+tanh
    dmacopy=1,         # 1: place h1 into S1next via SBUF->SBUF DMA on the idle
                       # SP HW-DGE ring (read one timestep later -> latency-free)
                       # instead of DVE copies. GPSIMD is NOT an option: its
                       # ops run ~2us each AND throttle concurrent 2-port DVE
                       # ops ~2.5x via the shared 2nd SBUF port (measured).
)
for _k in list(CFG):
    _v = os.environ.get(f"KCFG_{_k.upper()}")
    if _v is not None:
        CFG[_k] = int(_v) if _v.isdigit() else _v
if CFG["c_dtype"] == "f32" and CFG["s1_depth"] > 2:
    CFG["s1_depth"] = 2   # SBUF budget


def _fold_weights(inputs):
    """Host-side exact algebraic folding (all fp32 numpy)."""
    f = np.float32
    W_red = inputs["W_red"].astype(f)
    b_red = inputs["b_red"].astype(f)
    # de-normalization of channels 11 (u) and 12 (v), folded into W_red
    a = np.ones(CIN, f); a[11] = f(0.15); a[12] = f(0.12)
    d = np.zeros(CIN, f); d[11] = f(0.02); d[12] = f(-0.01)
    W_red_eff = W_red * a[None, :]
    b_red_eff = b_red + W_red @ d

    W1x = inputs["Wih1"].astype(f) @ W_red_eff          # [256, 28]
    W1h = inputs["Whh1"].astype(f)                      # [256, 64]
    b1 = (inputs["bih1"] + inputs["bhh1"]).astype(f) + inputs["Wih1"].astype(f) @ b_red_eff
    W21 = inputs["Wih2"].astype(f) @ inputs["Wc1"].astype(f)   # [256, 64]
    W22 = inputs["Whh2"].astype(f)                      # [256, 64]
    b2 = (inputs["bih2"] + inputs["bhh2"]).astype(f) + inputs["Wih2"].astype(f) @ inputs["bc1"].astype(f)
    whead = (inputs["W_head"].astype(f) @ inputs["Wc2"].astype(f))[0]     # [64]
    bhead = float((inputs["W_head"].astype(f) @ inputs["bc2"].astype(f) + inputs["b_head"].astype(f)).reshape(()))

    # reorder gate blocks (i, f, g, o) -> (i, f, o, g) so the three
    # sigmoid gates are contiguous in the PSUM tile
    perm = np.r_[0:64, 64:128, 192:256, 128:192]
    W1x, W1h, W21, W22 = W1x[perm], W1h[perm], W21[perm], W22[perm]
    b1, b2 = b1[perm], b2[perm]

    w1 = np.zeros((128, 256), f)
    w1[0:HID] = W1h.T
    w1[HID:HID + CIN] = W1x.T
    w1[HID + CIN] = b1
    # combined cell2 weight: rows 0:64 read h1 (V rows 0:64), rows
    # 64:128 read h2 (V rows 64:128) -- one K=128 matmul per gate-half
    w2 = np.zeros((128, 256), f)
    w2[0:HID] = W21.T
    w2[HID:128] = W22.T
    # cell2 bias rides an extra K=29 matmul against S1next's [x; ones]
    # rows only when nonzero (it is zero for the reference weights)
    w2c = None
    if np.any(b2 != 0):
        w2c = np.zeros((128, 256), f)
        w2c[HID + CIN] = b2
    wh = np.zeros((128, 1), f)
    wh[HID:, 0] = whead
    d = dict(w1=w1, w2=w2, wh=wh)
    if w2c is not None:
        d["w2c"] = w2c
    if CFG["gmode"] == "vtanh":
        # sigma(x) = (tanh(x/2) + 1)/2: halve the i,f,o gate pre-activations
        # (cols 0:192) so ONE Tanh inst covers all four gates; h is stored
        # doubled (h_hat = (tanh(o-pre)+1)*tanh(c)) so halve every weight row
        # that reads it; c is stored doubled too (ACT tanh(c) uses scale=0.5).
        for nm, M in d.items():
            if nm != "wh":
                M[:, 0:192] *= 0.5
        d["w1"][0:HID] *= 0.5     # rows reading h1_hat
        d["w2"][0:HID] *= 0.5     # rows reading h1_hat
        d["w2"][HID:128] *= 0.5   # rows reading h2_hat
        d["wh"] = wh * 0.5
    return d, bhead


def build(nc, bhead, has_b2):
    fd = CFG["fd"]
    ngrp = HALF // fd
    NSLOT = T * ngrp
    ND = CFG["s1_depth"]
    st_dt = {"f32": F32, "bf16": BF16}[CFG["c_dtype"]]

    x_d = nc.dram_tensor("xt", [T, CIN + 1, HW], BF16, kind="ExternalInput").ap()
    w_names = ["w1", "w2", "wh"] + (["w2c"] if has_b2 else [])
    w_dram = {nm: nc.dram_tensor(nm, [128, 1] if nm == "wh" else [128, 256], F32,
                                 kind="ExternalInput").ap() for nm in w_names}
    # out[i, j] = pixel j*128 + i of this core's [H, W] map (host transposes)
    out_d = nc.dram_tensor("out", [128, HW // 128], F32, kind="ExternalOutput").ap()

    with tile.TileContext(nc) as tc:
        with (
            tc.tile_pool(name="const", bufs=1) as const,
            tc.tile_pool(name="state", bufs=1) as state,
            tc.tile_pool(name="planes", bufs=CFG["plane_bufs"]) as planes,
            tc.tile_pool(name="hplanes", bufs=4) as hplanes,
            tc.tile_pool(name="outp", bufs=1) as outp,
            tc.tile_pool(name="psum", bufs=1, space=bass.MemorySpace.PSUM) as psum,
        ):
            # Stage weights via fp32 tiles + one convert copy each, so every
            # matmul waits on a single compute producer.
            w_sb = {}
            for nm in w_names:
                shp = [128, 1] if nm == "wh" else [128, 256]
                wf = const.tile(shp, F32, tag=f"{nm}f")
                nc.sync.dma_start(wf[:], w_dram[nm])
                wb = const.tile(shp, BF16, tag=nm)
                nc.vector.tensor_copy(wb[:], wf[:])
                w_sb[nm] = wb

            Ss = [state.tile([128, HW], BF16, tag=f"S1_{i}", name=f"S1_{i}")
                  for i in range(ND)]
            V = state.tile([128, HW], BF16, tag="V")
            c1 = state.tile([128, HALF], st_dt, tag="c1")
            c2 = state.tile([128, HALF], st_dt, tag="c2")
            out_sb = outp.tile([128, HW // 128], F32, tag="osb")

            # h2(-1) = 0 for cell2's first K=128 matmul. GPSIMD: slow (~50us)
            # but fully hidden behind the first x-frame DMA, and it keeps the
            # 13.7us DVE memset off the warm-up critical path.
            nc.gpsimd.memset(V[HID:128, :], 0.0)

            # x frame DMAs. Frame f lands in Ss[f % ND] rows 64:93. The
            # first frame is split into column-quarters ordered (A0, B0,
            # A1, B1) so slot (0, 0) can start after ~2 quarters instead
            # of the whole 950KB transfer.
            QW = HW // 4

            # x frames ride the Activation HW-DGE ring (one ~700ns trigger per
            # timestep on the Scalar queue) so the SP ring stays free for the
            # latency-sensitive h1 placement copies -- the rings are in-order,
            # and a 36us frame transfer ahead of a copy stalls the pipeline.
            xdma = nc.scalar if CFG["dmacopy"] else nc.sync

            def dma_frame(f, split):
                dst = Ss[f % ND]
                if split:
                    for q in (0, 2, 1, 3):
                        xdma.dma_start(dst[HID:K1, q * QW:(q + 1) * QW],
                                       x_d[f][:, q * QW:(q + 1) * QW])
                else:
                    xdma.dma_start(dst[HID:K1, :], x_d[f])

            dma_frame(0, split=True)
            dma_frame(1, split=True)

            # per-slot live tile handles (psum gate tiles + act output planes)
            P0s, P1s, pl1, pl2 = {}, {}, {}, {}

            def slot_tg(s):
                return s // ngrp, s % ngrp

            def cols(g):
                return g * fd, HALF + g * fd      # A-half / B-half col starts

            def c1_mms(s):
                t, g = slot_tg(s)
                Scur = Ss[t % ND]
                if g == 0 and t + 2 < T:
                    dma_frame(t + 2, split=False)
                a0, b0 = cols(g)
                ks = slice(0, K1) if t > 0 else slice(HID, K1)
                P = psum.tile([128, 4 * fd], F32, tag="P0", name="P0")
                P0s[s] = P
                for q in range(4):
                    for (cb, po) in ((a0, 0), (b0, 64)):
                        nc.tensor.matmul(
                            P[po:po + 64, q * fd:(q + 1) * fd],
                            w_sb["w1"][ks, q * 64:(q + 1) * 64],
                            Scur[ks, cb:cb + fd],
                        )

            def act_gates(s, Ps, pl, tagp):
                P = Ps.pop(s)
                if CFG["gmode"] == "vtanh":
                    sfo = planes.tile([128, 4 * fd], BF16, tag=f"sfo{tagp}")
                    nc.scalar.activation(sfo[:], P[:], AF.Tanh)
                    pl[s] = (sfo, sfo[:, 3 * fd:4 * fd])
                else:
                    sfo = planes.tile([128, 3 * fd], BF16, tag=f"sfo{tagp}")
                    tgp = planes.tile([128, fd], BF16, tag=f"tg{tagp}")
                    nc.scalar.activation(sfo[:], P[:, 0:3 * fd], AF.Sigmoid)
                    nc.scalar.activation(tgp[:], P[:, 3 * fd:4 * fd], AF.Tanh)
                    pl[s] = (sfo, tgp[:])

            def act1(s):
                act_gates(s, P0s, pl1, "1")

            def dve_c(s, pl, cc, tagp, eng):
                # gmode=vtanh (v* = tanh(pre/2), c_hat = 2c):
                #   c_hat = (v_f+1)*c_hat*0.5 + (v_i+1)*tanh(g)
                # gmode=tanh: c = sig(f)*c + sig(i)*tanh(g)
                t, g = slot_tg(s)
                sfo, tgp = pl[s]
                cg = slice(g * fd, (g + 1) * fd)
                si = sfo[:, 0:fd]
                sf = sfo[:, fd:2 * fd]
                Alu = mybir.AluOpType
                if CFG["gmode"] == "vtanh":
                    if t > 0:
                        p = planes.tile([128, fd], BF16, tag=f"t2{tagp}")
                        q = planes.tile([128, fd], st_dt, tag=f"t1{tagp}")
                        eng.scalar_tensor_tensor(
                            p[:], si, 1.0, tgp, Alu.add, Alu.mult)
                        eng.scalar_tensor_tensor(
                            q[:], sf, 1.0, cc[:, cg], Alu.add, Alu.mult)
                        eng.scalar_tensor_tensor(
                            cc[:, cg], q[:], 0.5, p[:], Alu.mult, Alu.add)
                    else:
                        eng.scalar_tensor_tensor(
                            cc[:, cg], si, 1.0, tgp, Alu.add, Alu.mult)
                    return
                t2 = planes.tile([128, fd], BF16, tag=f"t2{tagp}")
                if t > 0:
                    t1 = planes.tile([128, fd], st_dt, tag=f"t1{tagp}")
                    eng.tensor_mul(t2[:], si, tgp)
                    eng.tensor_mul(t1[:], sf, cc[:, cg])
                    eng.tensor_add(cc[:, cg], t1[:], t2[:])
                else:
                    eng.tensor_mul(cc[:, cg], si, tgp)

            def act_tc(s, pl, cc, tagp):
                t, g = slot_tg(s)
                cg = slice(g * fd, (g + 1) * fd)
                tch = planes.tile([128, fd], BF16, tag=f"tch{tagp}")
                scl = 0.5 if CFG["gmode"] == "vtanh" else 1.0
                nc.scalar.activation(tch[:], cc[:, cg], AF.Tanh, scale=scl)
                pl[s] = (pl[s][0], tch)

            def h_muls(s, pl, dsts, tagp, eng):
                # h = sig(o)*tanh(c); vtanh stores h_hat = (v_o+1)*tanh(c) = 2h
                # dsts: list of (tile, row_slice) to place both halves into.
                # The STT can run on DVE or GPSIMD; the placement copies
                # stay on DVE (partition-shifting, which Q7 cores cannot do).
                t, g = slot_tg(s)
                sfo, tch = pl.pop(s)
                a0, b0 = cols(g)
                Alu = mybir.AluOpType
                if CFG["gmode"] == "vtanh":
                    # one full-width STT then 4x-mode bf16 copies into the
                    # state rows (1 x ~424ns + 2N x ~195ns)
                    # deep pool: the S1next placement DMA reads hp with multi-us
                    # latency; 4 bufs keep the STT from stalling on the WAR
                    hp = hplanes.tile([128, fd], BF16, tag=f"hp{tagp}")
                    eng.scalar_tensor_tensor(
                        hp[:], sfo[:, 2 * fd:3 * fd], 1.0, tch[:],
                        Alu.add, Alu.mult)
                    for dst, rows, via_dma in dsts:
                        if via_dma:
                            nc.sync.dma_start(dst[rows, a0:a0 + fd], hp[0:64, :])
                            nc.sync.dma_start(dst[rows, b0:b0 + fd], hp[64:128, :])
                        else:
                            nc.vector.tensor_copy(dst[rows, a0:a0 + fd], hp[0:64, :])
                            nc.vector.tensor_copy(dst[rows, b0:b0 + fd], hp[64:128, :])
                else:
                    for dst, rows, _ in dsts:
                        for (po, cb) in ((0, a0), (64, b0)):
                            so = sfo[po:po + 64, 2 * fd:3 * fd]
                            eng.tensor_mul(dst[rows, cb:cb + fd],
                                           so, tch[po:po + 64, :])

            def c2_mms(s):
                t, g = slot_tg(s)
                a0, b0 = cols(g)
                P = psum.tile([128, 4 * fd], F32, tag="P1", name="P1")
                P1s[s] = P
                # ONE K=128 mm per gate-half: rows 0:64 of V = h1(t) (just
                # written by h_muls this slot), rows 64:128 = h2(t-1).
                # Alternating column halves -> col-tiled pairs run
                # concurrently on the PE's 64-col groups.
                halves = ((a0, 0), (b0, 64))
                for wave in range(2):
                    for q in range(4):
                        cb, po = halves[(q + wave) % 2]
                        nc.tensor.matmul(
                            P[po:po + 64, q * fd:(q + 1) * fd],
                            w_sb["w2"][0:128, q * 64:(q + 1) * 64],
                            V[0:128, cb:cb + fd],
                            start=True, stop=not has_b2,
                        )
                        if has_b2:
                            Snxt = Ss[(t + 1) % ND]
                            nc.tensor.matmul(
                                P[po:po + 64, q * fd:(q + 1) * fd],
                                w_sb["w2c"][HID:K1, q * 64:(q + 1) * 64],
                                Snxt[HID:K1, cb:cb + fd],
                                start=False, stop=True,
                            )

            def act2(s):
                act_gates(s, P1s, pl2, "2")

            # Software pipeline with cell2 lagged ONE slot: the per-engine
            # queues are in-order, so emission order IS execution order. The
            # lag interleaves acts as [act1(s), act2(s-1), tc1(s), tc2(s-1)]
            # -- every wait (DVE chain of s, matmuls of s-1) overlaps with
            # work of the other slot instead of idling the ACT queue.
            def cell1_front(s):
                t, g = slot_tg(s)
                c1_mms(s)
                act1(s)

            def cell1_back(s):
                t, g = slot_tg(s)
                S1next = Ss[(t + 1) % ND]
                dve_c(s, pl1, c1, "a", nc.vector)
                act_tc(s, pl1, c1, "a")
                # V placement first so cell2's matmuls release earliest;
                # S1next placement is dead at the last timestep and is only
                # read one timestep later, so it can ride the idle DMA ring.
                d1 = [(V, slice(0, HID), False)]
                if t + 1 < T:
                    d1.append((S1next, slice(0, HID), bool(CFG["dmacopy"])))
                h_muls(s, pl1, d1, 'a', nc.vector)

            def cell2_front(s):
                c2_mms(s)
                act2(s)

            def cell2_back(s):
                dve_c(s, pl2, c2, "b", nc.vector)
                act_tc(s, pl2, c2, "b")
                h_muls(s, pl2, [(V, slice(HID, 128), False)], 'b', nc.vector)

            for s in range(NSLOT):
                cell1_front(s)
                if s > 0:
                    cell2_front(s - 1)
                cell1_back(s)
                if s > 0:
                    cell2_back(s - 1)
            cell2_front(NSLOT - 1)
            cell2_back(NSLOT - 1)

            # head: out[pix] = whead @ h2[pix] + bhead, pixels as matmul M-dim
            ncols = HW // 128
            ph = psum.tile([128, ncols], F32, tag="P0", name="ph")
            for j in range(ncols):
                nc.tensor.matmul(
                    ph[:, j:j + 1],
                    V[HID:128, j * 128:(j + 1) * 128],
                    w_sb["wh"][HID:128, 0:1],
                )
            nc.vector.tensor_scalar_add(out_sb[:], ph[:], float(bhead))
            nc.sync.dma_start(out_d, out_sb[:])
    nc.compile()
    return nc


def _make_nc():
    # Bacc (not raw Bass): its compile() runs move_matmul_waits_to_ldweights +
    # generate_event_semaphores, required to satisfy TRN2's 1-wait-per-inst limit.
    return bacc.Bacc("TRN2", target_bir_lowering=False, debug=False,
                     num_devices=NCORES, enable_partition_id=False)


def _in_maps(inputs):
    folded, _ = _fold_weights(inputs)
    x = np.asarray(inputs["x"], dtype=np.float32)
    x_bf = x.reshape(NCORES, T, CIN, HW).astype(ml_dtypes.bfloat16)
    ones = np.ones((T, 1, HW), ml_dtypes.bfloat16)
    maps = []
    for b in range(NCORES):
        m = dict(folded)
        m["xt"] = np.ascontiguousarray(
            np.concatenate([x_bf[b], ones], axis=1))
        maps.append(m)
    return maps


def _assemble(results):
    out = np.empty((NCORES, H, W), np.float32)
    for b in range(NCORES):
        o = results[b]["out"]          # [128, HW//128], o[i, j] = pixel j*128+i
        out[b] = o.T.reshape(H, W)
    return out


def _run(inputs, trace=False):
    folded, bhead = _fold_weights(inputs)
    nc = build(_make_nc(), bhead, "w2c" in folded)
    maps = _in_maps(inputs)
    res = run_bass_kernel_spmd(nc, maps, core_ids=list(range(NCORES)), trace=trace)
    return _assemble(res.results), res


def kernel(**inputs) -> np.ndarray:
    out, _ = _run(inputs, trace=False)
    return out
